# revision 1
# baseline (speedup 1.0000x reference)
"""Trainium2 Bass kernel for nn_NodeModel (GNN message passing).

agg = segment_sum(x[row], col, N); h = [x, agg]; out = MLP(h)

Strategy (8 NeuronCores, SPMD):
  - Host: degree-bucket the destination nodes, round-robin nodes of each
    degree across the 8 cores (load balance), build per-core int32 slot
    tables (one slot = one in-edge's source row id, padded with a zero row).
  - Device per core: For_i hardware loop of indirect DMA gathers
    (128 rows x 16B per instruction) streaming x[row] slot values to a
    DRAM staging buffer; static phase re-reads staging per degree bucket,
    DVE-reduces each node's d slots, concats with x, runs the 3-layer MLP
    on the tensor engine via 8-node block-diagonal weights.
  - Host: inverse-permute the per-core outputs into the full [N, 3] array.

Self-contained: hardcodes problem shapes; requires only numpy + concourse.
"""

import numpy as np

N_NODES = 1_000_000
N_EDGES = 26_000_000
N_CORES = 8
P = 128
U = 1024  # gather unroll per For_i iteration
CHUNK_SLOTS = 2048  # max per-partition slots per reduce chunk (fp32 quads)


def _ceil_to(a, m):
    return (a + m - 1) // m * m


def _host_prep(x, row, col):
    """Build per-core layouts. Returns dict with device inputs + inverse map."""
    N = x.shape[0]
    ZROW = N  # index of the zero row in x_dev

    deg = np.bincount(col, minlength=N).astype(np.int64)
    order = np.argsort(col, kind="stable")
    sorted_rows = row[order].astype(np.int32)
    node_start = np.zeros(N + 1, np.int64)
    np.cumsum(deg, out=node_start[1:])

    # merge sparse degree buckets upward (extra slots masked to ZROW) to
    # cut per-bucket partition-row padding, then round-robin across cores
    degs_present = np.unique(deg)
    lut = np.arange(int(degs_present.max()) + 1)
    cnt = {int(dv): int((deg == dv).sum()) for dv in degs_present}
    group, gn = [], 0
    for dv in [int(v) for v in degs_present if v > 0]:
        group.append(dv)
        gn += cnt[dv]
        if gn >= 96 * N_CORES:
            for g in group:
                lut[g] = dv
            group, gn = [], 0
    for g in group:
        lut[g] = group[-1]
    deg_eff = lut[deg]
    degs_present = np.unique(deg_eff)
    # per (effective degree): list of nodes per core
    per_core_nodes = {d: [] for d in degs_present}
    for d in degs_present:
        nodes_d = np.flatnonzero(deg_eff == d)
        for k in range(N_CORES):
            per_core_nodes[d].append(nodes_d[k::N_CORES])

    # shared bucket structure (same for all cores)
    buckets = []  # (d, m_d) with N_d = 128*m_d nodes per core
    for d in degs_present:
        n_max = max(len(per_core_nodes[d][k]) for k in range(N_CORES))
        N_d = _ceil_to(max(n_max, 1), P)
        buckets.append((int(d), N_d // P))

    J = sum(m for _, m in buckets)
    J = _ceil_to(J, 16)  # MLP slab alignment

    # chunk schedule: list of (d, m_c, j_off, slot_off, W_c) shared by cores
    chunks = []
    j_off = 0
    slot_off = 0  # per-partition slot offset into the stream
    for d, m_d in buckets:
        if d == 0:
            j_off += m_d
            continue
        m_max = max(1, CHUNK_SLOTS // d)
        done = 0
        while done < m_d:
            m_c = min(m_max, m_d - done)
            W_c = _ceil_to(m_c * d, 4)
            chunks.append((d, m_c, j_off + done, slot_off, W_c))
            slot_off += W_c
            done += m_c
        j_off += m_d
    W_tot = _ceil_to(max(slot_off, U), U)
    ITERS = W_tot // U

    # build per-core slot streams, x_shard, inverse map
    streams = np.full((N_CORES, P, W_tot), ZROW, np.int32)
    x_shard = np.zeros((N_CORES, P, J, 3), np.float32)
    node_core = np.zeros(N, np.int32)
    node_p = np.zeros(N, np.int32)
    node_j = np.zeros(N, np.int32)

    # per-core cursor per degree bucket start
    for k in range(N_CORES):
        # map (d) -> j offset of bucket start
        j_cursor = 0
        bucket_joff = {}
        for d, m_d in buckets:
            bucket_joff[d] = j_cursor
            j_cursor += m_d
        for d, m_d in buckets:
            arr = per_core_nodes[d][k]
            n = len(arr)
            if n == 0:
                continue
            i = np.arange(n)
            pp = i // m_d
            jj = i % m_d
            jglob = bucket_joff[d] + jj
            node_core[arr] = k
            node_p[arr] = pp
            node_j[arr] = jglob
            x_shard[k, pp, jglob] = x[arr]
        # fill slots chunk by chunk
        for d, m_c, jc_off, s_off, W_c in chunks:
            joff0 = bucket_joff[d]
            # nodes of this chunk: bucket-node index range per partition
            arr = per_core_nodes[d][k]
            n = len(arr)
            if n == 0:
                continue  # stream stays ZROW padding
            m_d = dict(buckets)[d]
            # chunk covers bucket-local j range [jc_off-joff0, +m_c)
            jlo = jc_off - joff0
            # node at (p, jlo+t) has bucket-node index p*m_d + jlo + t
            pi = np.arange(P)[:, None]
            ti = np.arange(m_c)[None, :]
            bidx = pi * m_d + jlo + ti  # [P, m_c]
            valid = bidx < n
            nodes = np.where(valid, np.take(arr, np.minimum(bidx, max(n - 1, 0))), -1)
            starts = np.where(nodes >= 0, node_start[np.maximum(nodes, 0)], 0)
            sl = starts[:, :, None] + np.arange(d)[None, None, :]  # [P, m_c, d]
            vals = sorted_rows[np.minimum(sl, len(sorted_rows) - 1)]
            nd = np.where(nodes >= 0, deg[np.maximum(nodes, 0)], 0)
            smask = np.arange(d)[None, None, :] < nd[:, :, None]
            vals = np.where(smask, vals, ZROW).astype(np.int32)
            streams[k, :, s_off : s_off + m_c * d] = vals.reshape(P, m_c * d)

    offs = streams.reshape(N_CORES, P, ITERS, U).transpose(0, 2, 1, 3).copy()

    x_dev = np.zeros((N + 1, 4), np.float32)
    x_dev[:N, :3] = x

    return dict(
        offs=offs, x_dev=x_dev, x_shard=x_shard, chunks=chunks, J=J,
        ITERS=ITERS, W_tot=W_tot,
        node_core=node_core, node_p=node_p, node_j=node_j,
    )


def _build_program(prep, W1, b1, W2, b2, W3, b3):
    import concourse.bass as bass
    import concourse.bacc as bacc
    import concourse.mybir as mybir
    import concourse.tile as tile

    J = prep["J"]
    ITERS = prep["ITERS"]
    chunks = prep["chunks"]
    NV = prep["x_dev"].shape[0]

    # block-diagonal weights (8 nodes packed)
    W1bd = np.zeros((48, 128), np.float32)
    W2bd = np.zeros((128, 128), np.float32)
    W3bd = np.zeros((128, 24), np.float32)
    for n in range(8):
        W1bd[n * 6 : n * 6 + 6, n * 16 : n * 16 + 16] = W1
        W2bd[n * 16 : n * 16 + 16, n * 16 : n * 16 + 16] = W2
        W3bd[n * 16 : n * 16 + 16, n * 3 : n * 3 + 3] = W3
    b1t = np.tile(b1, 8).astype(np.float32)[:, None]
    b2t = np.tile(b2, 8).astype(np.float32)[:, None]
    b3t = np.tile(b3, 8).astype(np.float32)[:, None]

    nc = bacc.Bacc("TRN2", target_bir_lowering=False, debug=False,
                   num_devices=N_CORES, dynamic_dma_scratch_size=49152)
    f32, i32 = mybir.dt.float32, mybir.dt.int32
    x_d = nc.dram_tensor("x_dev", [NV, 4], f32, kind="ExternalInput").ap()
    offs_d = nc.dram_tensor("offs", [ITERS, P, U], i32, kind="ExternalInput").ap()
    xs_d = nc.dram_tensor("x_shard", [P, J * 3], f32, kind="ExternalInput").ap()
    w1_d = nc.dram_tensor("W1bd", [48, 128], f32, kind="ExternalInput").ap()
    w2_d = nc.dram_tensor("W2bd", [128, 128], f32, kind="ExternalInput").ap()
    w3_d = nc.dram_tensor("W3bd", [128, 24], f32, kind="ExternalInput").ap()
    b1_d = nc.dram_tensor("b1t", [128, 1], f32, kind="ExternalInput").ap()
    b2_d = nc.dram_tensor("b2t", [128, 1], f32, kind="ExternalInput").ap()
    b3_d = nc.dram_tensor("b3t", [24, 1], f32, kind="ExternalInput").ap()
    out_d = nc.dram_tensor("out", [P, J * 3], f32, kind="ExternalOutput").ap()
    W_tot = prep["W_tot"]
    stag = nc.dram_tensor("stag", [P, W_tot * 4], f32).ap()

    with tile.TileContext(nc) as tc:
        with (
            tc.tile_pool(name="op", bufs=3) as op,
            tc.tile_pool(name="gp", bufs=3) as gp,
            tc.tile_pool(name="rp", bufs=2) as rp,
            tc.tile_pool(name="hp", bufs=1) as hp,
            tc.tile_pool(name="wp", bufs=1) as wp,
            tc.tile_pool(name="mp", bufs=5) as mp,
            tc.tile_pool(name="pp", bufs=2, space="PSUM") as pp,
            tc.tile_pool(name="pq", bufs=2, space="PSUM") as pq,
        ):
            # ---- phase 1: gather loop ----
            with tc.For_i(0, ITERS, 1, hint_engines=(mybir.EngineType.Pool,)) as i:
                ot = op.tile([P, U], i32)
                nc.sync.dma_start(out=ot[:], in_=offs_d[i, :, :])
                g = gp.tile([P, U * 4], f32)
                for u in range(U):
                    nc.gpsimd.indirect_dma_start(
                        out=g[:, u * 4 : (u + 1) * 4],
                        out_offset=None,
                        in_=x_d[:],
                        in_offset=bass.IndirectOffsetOnAxis(ap=ot[:, u : u + 1], axis=0),
                    )
                nc.sync.dma_start(out=stag[:, bass.ts(i, U * 4)], in_=g[:])

            # ---- phase 2: reduce per bucket chunk ----
            h = hp.tile([P, J * 6], f32, tag="h")
            nc.vector.memset(h[:], 0.0)
            hv = h[:].rearrange("p (j c) -> p j c", c=6)
            # x part
            xs = hp.tile([P, J * 3], f32, tag="xs")
            nc.sync.dma_start(out=xs[:], in_=xs_d[:])
            nc.vector.tensor_copy(
                out=hv[:, :, 0:3],
                in_=xs[:].rearrange("p (j c) -> p j c", c=3),
            )
            for d, m_c, j0, s_off, W_c in chunks:
                gt = rp.tile([P, m_c * d * 4], f32, tag="gt")
                nc.sync.dma_start(
                    out=gt[:], in_=stag[:, s_off * 4 : (s_off + m_c * d) * 4]
                )
                gv = gt[:, : m_c * d * 4].rearrange(
                    "p (m d c) -> p m c d", d=d, c=4
                )
                nc.vector.reduce_sum(
                    out=hv[:, j0 : j0 + m_c, 3:6],
                    in_=gv[:, :, 0:3, :],
                    axis=mybir.AxisListType.X,
                )

            # ---- phase 3: MLP ----
            w1t = wp.tile([48, 128], f32, tag="w1")
            w2t = wp.tile([128, 128], f32, tag="w2")
            w3t = wp.tile([128, 24], f32, tag="w3")
            bt1 = wp.tile([128, 1], f32, tag="b1")
            bt2 = wp.tile([128, 1], f32, tag="b2")
            bt3 = wp.tile([24, 1], f32, tag="b3")
            nc.sync.dma_start(out=w1t[:], in_=w1_d[:])
            nc.sync.dma_start(out=w2t[:], in_=w2_d[:])
            nc.sync.dma_start(out=w3t[:], in_=w3_d[:])
            nc.sync.dma_start(out=bt1[:], in_=b1_d[:])
            nc.sync.dma_start(out=bt2[:], in_=b2_d[:])
            nc.sync.dma_start(out=bt3[:], in_=b3_d[:])
            ident = wp.tile([P, P], f32, tag="id")
            from concourse.masks import make_identity

            make_identity(nc, ident[:])

            outv = out_d.rearrange("p jc -> jc p")
            for slab in range(J // 16):
                jbase = slab * 16
                for half in range(2):
                    jb = jbase + half * 8
                    pt = pp.tile([48, 128], f32, tag="pt", space="PSUM")
                    nc.tensor.transpose(
                        out=pt[:],
                        in_=h[:, jb * 6 : (jb + 8) * 6],
                        identity=ident[:],
                    )
                    sbT = mp.tile([48, 128], f32, tag="sbT")
                    nc.scalar.activation(
                        sbT[:], pt[:], mybir.ActivationFunctionType.Identity,
                        bias=0.0,
                    )
                    rhs = sbT[:]
                    ps1 = pq.tile([128, 128], f32, tag="ps1", space="PSUM")
                    nc.tensor.matmul(ps1[:], lhsT=w1t[:], rhs=rhs, start=True, stop=True)
                    s1 = mp.tile([128, 128], f32, tag="s1")
                    nc.scalar.activation(
                        s1[:], ps1[:], mybir.ActivationFunctionType.Relu,
                        bias=bt1[:, 0:1],
                    )
                    ps2 = pq.tile([128, 128], f32, tag="ps2", space="PSUM")
                    nc.tensor.matmul(ps2[:], lhsT=w2t[:], rhs=s1[:], start=True, stop=True)
                    s2 = mp.tile([128, 128], f32, tag="s2")
                    nc.scalar.activation(
                        s2[:], ps2[:], mybir.ActivationFunctionType.Relu,
                        bias=bt2[:, 0:1],
                    )
                    ps3 = pq.tile([24, 128], f32, tag="ps3", space="PSUM")
                    nc.tensor.matmul(ps3[:], lhsT=w3t[:], rhs=s2[:], start=True, stop=True)
                    s3 = mp.tile([24, 128], f32, tag="s3")
                    nc.scalar.activation(
                        s3[:], ps3[:], mybir.ActivationFunctionType.Identity,
                        bias=bt3[:, 0:1],
                    )
                    j0 = (jbase + half * 8) * 3
                    nc.sync.dma_start(out=outv[j0 : j0 + 24, :], in_=s3[:])

    nc.compile()
    consts = dict(W1bd=W1bd, W2bd=W2bd, W3bd=W3bd, b1t=b1t, b2t=b2t, b3t=b3t)
    return nc, consts


def kernel(x, edge_index, edge_attr, u, batch, W1, b1, W2, b2, W3, b3):
    import sys, types

    # inject antenv.axon_hooks if missing (NTFF profiling under axon)
    try:
        import antenv.axon_hooks  # noqa: F401
    except ImportError:
        try:
            import antenv
            mod = types.ModuleType("antenv.axon_hooks")
            _H = [None]
            mod.set_axon_ntff_profile_hook = lambda h: _H.__setitem__(0, h)
            mod.get_axon_ntff_profile_hook = lambda: _H[0]
            sys.modules["antenv.axon_hooks"] = mod
            antenv.axon_hooks = mod
            from trn_agent_boot.trn_boot import _ntff_profile_via_ctypes
            hook = _ntff_profile_via_ctypes("/opt/axon/libaxon_pjrt.so")
            if hook is not None:
                mod.set_axon_ntff_profile_hook(hook)
        except Exception:
            pass

    from concourse.bass_utils import run_bass_kernel_spmd

    x = np.asarray(x, np.float32)
    row = np.asarray(edge_index[0]).astype(np.int64)
    col = np.asarray(edge_index[1]).astype(np.int64)
    W1 = np.asarray(W1, np.float32); b1 = np.asarray(b1, np.float32)
    W2 = np.asarray(W2, np.float32); b2 = np.asarray(b2, np.float32)
    W3 = np.asarray(W3, np.float32); b3 = np.asarray(b3, np.float32)

    prep = _host_prep(x, row, col)
    nc, consts = _build_program(prep, W1, b1, W2, b2, W3, b3)

    in_maps = []
    for k in range(N_CORES):
        in_maps.append({
            "x_dev": prep["x_dev"],
            "offs": prep["offs"][k],
            "x_shard": prep["x_shard"][k].reshape(P, -1),
            "W1bd": consts["W1bd"], "W2bd": consts["W2bd"], "W3bd": consts["W3bd"],
            "b1t": consts["b1t"], "b2t": consts["b2t"], "b3t": consts["b3t"],
        })
    import os
    _tr = os.environ.get("KERNEL_TRACE") == "1"
    res = run_bass_kernel_spmd(nc, in_maps, list(range(N_CORES)), trace=_tr)
    if _tr:
        kernel._last_exec_ns = res.exec_time_ns
    J = prep["J"]
    outs = np.stack([res.results[k]["out"].reshape(P, J, 3) for k in range(N_CORES)])
    full = outs[prep["node_core"], prep["node_p"], prep["node_j"]]
    kernel._last_nc = nc
    kernel._last_in_maps = in_maps
    return full.astype(np.float32)



# revision 9
# speedup vs baseline: 15.8891x; 15.8891x over previous
"""Trainium2 Bass kernel for nn_NodeModel (GNN message passing).

agg = segment_sum(x[row], col, N); h = [x, agg]; out = MLP(h)

Strategy (8 NeuronCores, SPMD):
  - Host: degree-bucket destination nodes, round-robin across 8 cores,
    build per-core slot streams (slot = one in-edge's source row id,
    ZROW pad) packed into fixed-size gather windows.
  - Device per core: per window, ONE offsets DMA + a few multi-run
    indirect DMAs gather x[row] (12B rows into 16B cells) straight into
    SBUF; DVE reduce_sum per degree-chunk accumulates agg into h; 3-layer
    MLP on the tensor engine with 8-node block-diagonal weights, 512-wide
    rhs tiles.
  - Host: inverse-permute per-core outputs into the full [N, 3] array.

Self-contained: hardcodes problem shapes; requires only numpy + concourse.
"""

import numpy as np

N_NODES = 1_000_000
N_EDGES = 26_000_000
N_CORES = 8
P = 128
W = 2048          # gather-window slots per partition
CELL = 4          # f32 cell per slot in SBUF
ROWW = 4          # f32 payload per slot (x row padded to 16B)


def _ceil_to(a, m):
    return (a + m - 1) // m * m


def _host_prep(x, row, col):
    N = x.shape[0]
    ZROW = N

    deg = np.bincount(col, minlength=N).astype(np.int64)
    order = np.argsort(col, kind="stable")
    sorted_rows = row[order].astype(np.int32)
    node_start = np.zeros(N + 1, np.int64)
    np.cumsum(deg, out=node_start[1:])

    # merge sparse degree buckets upward (extra slots masked to ZROW)
    degs_present = np.unique(deg)
    lut = np.arange(int(degs_present.max()) + 1)
    cnt = {int(dv): int((deg == dv).sum()) for dv in degs_present}
    group, gn = [], 0
    for dv in [int(v) for v in degs_present if v > 0]:
        group.append(dv)
        gn += cnt[dv]
        if gn >= 96 * N_CORES:
            for g in group:
                lut[g] = dv
            group, gn = [], 0
    for g in group:
        lut[g] = group[-1]
    deg_eff = lut[deg]
    degs_present = [int(v) for v in np.unique(deg_eff) if v > 0]

    per_core_nodes = {}
    zero_nodes_core = []
    nodes0 = np.flatnonzero(deg_eff == 0)
    for k in range(N_CORES):
        zero_nodes_core.append(nodes0[k::N_CORES])
    for d in degs_present:
        nodes_d = np.flatnonzero(deg_eff == d)
        per_core_nodes[d] = [nodes_d[k::N_CORES] for k in range(N_CORES)]

    # bucket table: (d, m_d); nodes per core laid out [P, m_d]
    buckets = []
    for d in degs_present:
        n_max = max(len(per_core_nodes[d][k]) for k in range(N_CORES))
        buckets.append((d, _ceil_to(max(n_max, 1), P) // P))
    m0 = _ceil_to(max(len(a) for a in zero_nodes_core), P) // P if len(nodes0) else 0

    # window/chunk schedule (shared by all cores)
    # chunk: (d, m_c, j0, win, k0); windows are W slots per partition
    chunks = []
    j_cursor = m0  # deg-0 nodes take the first m0 j-columns
    bucket_j0 = {}
    win, k_cur = 0, 0
    for d, m_d in buckets:
        bucket_j0[d] = j_cursor
        rem = m_d
        while rem > 0:
            space = W - k_cur
            m_fit = min(rem, space // d)
            if m_fit == 0:
                win += 1
                k_cur = 0
                continue
            chunks.append((d, m_fit, j_cursor, win, k_cur))
            k_cur += m_fit * d
            j_cursor += m_fit
            rem -= m_fit
    NWIN = win + 1
    J = _ceil_to(j_cursor, 32)

    streams = np.full((N_CORES, NWIN, P, W), ZROW, np.int32)
    x_shard6 = np.zeros((N_CORES, P, J, 6), np.float32)
    node_core = np.zeros(N, np.int32)
    node_p = np.zeros(N, np.int32)
    node_j = np.zeros(N, np.int32)

    m_of = dict(buckets)
    for k in range(N_CORES):
        arr0 = zero_nodes_core[k]
        if len(arr0):
            i = np.arange(len(arr0))
            node_core[arr0] = k
            node_p[arr0] = i // m0
            node_j[arr0] = i % m0
            x_shard6[k, i // m0, i % m0, 0:3] = x[arr0]
        for d in degs_present:
            arr = per_core_nodes[d][k]
            n = len(arr)
            if n == 0:
                continue
            m_d = m_of[d]
            i = np.arange(n)
            pp_, jj = i // m_d, i % m_d
            jglob = bucket_j0[d] + jj
            node_core[arr] = k
            node_p[arr] = pp_
            node_j[arr] = jglob
            x_shard6[k, pp_, jglob, 0:3] = x[arr]
        for d, m_c, j0, w_, k0 in chunks:
            arr = per_core_nodes[d][k]
            n = len(arr)
            if n == 0:
                continue
            m_d = m_of[d]
            jlo = j0 - bucket_j0[d]
            pi_ = np.arange(P)[:, None]
            ti = np.arange(m_c)[None, :]
            bidx = pi_ * m_d + jlo + ti  # [P, m_c] bucket-node index
            valid = bidx < n
            nodes = np.where(valid, np.take(arr, np.minimum(bidx, max(n - 1, 0))), -1)
            starts = np.where(nodes >= 0, node_start[np.maximum(nodes, 0)], 0)
            sl = starts[:, :, None] + np.arange(d)[None, None, :]
            vals = sorted_rows[np.minimum(sl, len(sorted_rows) - 1)]
            nd = np.where(nodes >= 0, deg[np.maximum(nodes, 0)], 0)
            smask = np.arange(d)[None, None, :] < nd[:, :, None]
            vals = np.where(smask, vals, ZROW).astype(np.int32)
            streams[k, w_, :, k0 : k0 + m_c * d] = vals.reshape(P, m_c * d)

    # group chunks by window for the device loop
    win_chunks = [[] for _ in range(NWIN)]
    for d, m_c, j0, w_, k0 in chunks:
        win_chunks[w_].append((d, m_c, j0, k0))

    x_dev = np.zeros((N + 1, ROWW), np.float32)
    x_dev[:N, :3] = x

    return dict(
        streams=streams, x_dev=x_dev, x_shard6=x_shard6,
        win_chunks=win_chunks, J=J, NWIN=NWIN,
        node_core=node_core, node_p=node_p, node_j=node_j,
    )


def _build_program(prep, W1, b1, W2, b2, W3, b3):
    import concourse.bass as bass
    import concourse.bacc as bacc
    import concourse.mybir as mybir
    import concourse.tile as tile
    from concourse.masks import make_identity

    J = prep["J"]
    NWIN = prep["NWIN"]
    win_chunks = prep["win_chunks"]
    NV = prep["x_dev"].shape[0]

    W1bd = np.zeros((48, 128), np.float32)
    W2bd = np.zeros((128, 128), np.float32)
    W3bd = np.zeros((128, 24), np.float32)
    for n in range(8):
        W1bd[n * 6 : n * 6 + 6, n * 16 : n * 16 + 16] = W1
        W2bd[n * 16 : n * 16 + 16, n * 16 : n * 16 + 16] = W2
        W3bd[n * 16 : n * 16 + 16, n * 3 : n * 3 + 3] = W3
    b1t = np.tile(b1, 8).astype(np.float32)[:, None]
    b2t = np.tile(b2, 8).astype(np.float32)[:, None]
    b3t = np.tile(b3, 8).astype(np.float32)[:, None]

    nc = bacc.Bacc("TRN2", target_bir_lowering=False, debug=False,
                   num_devices=N_CORES, dynamic_dma_scratch_size=49152)
    f32, i32 = mybir.dt.float32, mybir.dt.int32
    x_d = nc.dram_tensor("x_dev", [NV, ROWW], f32, kind="ExternalInput").ap()
    offs_d = nc.dram_tensor("offs", [NWIN, P, W], i32, kind="ExternalInput").ap()
    h_d = nc.dram_tensor("h0", [P, J * 6], f32, kind="ExternalInput").ap()
    w1_d = nc.dram_tensor("W1bd", [48, 128], f32, kind="ExternalInput").ap()
    w2_d = nc.dram_tensor("W2bd", [128, 128], f32, kind="ExternalInput").ap()
    w3_d = nc.dram_tensor("W3bd", [128, 24], f32, kind="ExternalInput").ap()
    b1_d = nc.dram_tensor("b1t", [128, 1], f32, kind="ExternalInput").ap()
    b2_d = nc.dram_tensor("b2t", [128, 1], f32, kind="ExternalInput").ap()
    b3_d = nc.dram_tensor("b3t", [24, 1], f32, kind="ExternalInput").ap()
    out_d = nc.dram_tensor("out", [J * 3, P], f32, kind="ExternalOutput").ap()

    with tile.TileContext(nc) as tc:
        with (
            tc.tile_pool(name="op", bufs=2) as op,
            tc.tile_pool(name="gp", bufs=2) as gp,
            tc.tile_pool(name="hp", bufs=1) as hp,
            tc.tile_pool(name="wp", bufs=1) as wp,
            tc.tile_pool(name="mp", bufs=2) as mp,
            tc.tile_pool(name="pp", bufs=2, space="PSUM") as pp,
        ):
            # ---- h load (x interleaved, agg lanes zero) ----
            h = hp.tile([P, J * 6], f32, tag="h")
            nc.sync.dma_start(out=h[:], in_=h_d[:])
            hv = h[:].rearrange("p (j c) -> p j c", c=6)

            # ---- weights ----
            w1t = wp.tile([48, 128], f32, tag="w1")
            w2t = wp.tile([128, 128], f32, tag="w2")
            w3t = wp.tile([128, 24], f32, tag="w3")
            bt1 = wp.tile([128, 1], f32, tag="b1")
            bt2 = wp.tile([128, 1], f32, tag="b2")
            bt3 = wp.tile([24, 1], f32, tag="b3")
            nc.sync.dma_start(out=w1t[:], in_=w1_d[:])
            nc.sync.dma_start(out=w2t[:], in_=w2_d[:])
            nc.sync.dma_start(out=w3t[:], in_=w3_d[:])
            nc.sync.dma_start(out=bt1[:], in_=b1_d[:])
            nc.sync.dma_start(out=bt2[:], in_=b2_d[:])
            nc.sync.dma_start(out=bt3[:], in_=b3_d[:])
            ident = wp.tile([P, P], f32, tag="id")
            make_identity(nc, ident[:])

            # ---- gather + reduce per window ----
            for win in range(NWIN):
                ot = op.tile([P, W], i32)
                nc.sync.dma_start(out=ot[:], in_=offs_d[win, :, :])
                g = gp.tile([P, W * CELL], f32)
                for w_ in range(W):
                    nc.gpsimd.indirect_dma_start(
                        out=g[:, w_ * CELL : (w_ + 1) * CELL],
                        out_offset=None, in_=x_d[:],
                        in_offset=bass.IndirectOffsetOnAxis(
                            ap=ot[:, w_ : w_ + 1], axis=0))
                for d, m_c, j0, k0 in win_chunks[win]:
                    gvv = g[:, k0 * CELL : (k0 + m_c * d) * CELL].rearrange(
                        "p (m d c) -> p m c d", c=CELL, d=d)
                    nc.vector.reduce_sum(
                        out=hv[:, j0 : j0 + m_c, 3:6],
                        in_=gvv[:, :, 0:3, :],
                        axis=mybir.AxisListType.X)

            # ---- MLP: groups of 32 j-columns (4 half-slabs x 512 cols) ----
            for grp in range(J // 32):
                rhs = mp.tile([48, 512], f32, tag="rhs")
                for b in range(4):
                    jb = grp * 32 + b * 8
                    pt = pp.tile([48, 128], f32, tag="pt", space="PSUM")
                    nc.tensor.transpose(
                        out=pt[:],
                        in_=h[:, jb * 6 : (jb + 8) * 6],
                        identity=ident[:])
                    nc.scalar.activation(
                        rhs[:, b * 128 : (b + 1) * 128], pt[:],
                        mybir.ActivationFunctionType.Identity, bias=0.0)
                ps1 = pp.tile([128, 512], f32, tag="ps1", space="PSUM")
                nc.tensor.matmul(ps1[:], lhsT=w1t[:], rhs=rhs[:], start=True, stop=True)
                s1 = mp.tile([128, 512], f32, tag="s1")
                nc.scalar.activation(
                    s1[:], ps1[:], mybir.ActivationFunctionType.Relu, bias=bt1[:, 0:1])
                ps2 = pp.tile([128, 512], f32, tag="ps2", space="PSUM")
                nc.tensor.matmul(ps2[:], lhsT=w2t[:], rhs=s1[:], start=True, stop=True)
                s2 = mp.tile([128, 512], f32, tag="s2")
                nc.scalar.activation(
                    s2[:], ps2[:], mybir.ActivationFunctionType.Relu, bias=bt2[:, 0:1])
                ps3 = pp.tile([24, 512], f32, tag="ps3", space="PSUM")
                nc.tensor.matmul(ps3[:], lhsT=w3t[:], rhs=s2[:], start=True, stop=True)
                s3 = mp.tile([24, 512], f32, tag="s3")
                nc.scalar.activation(
                    s3[:], ps3[:], mybir.ActivationFunctionType.Identity,
                    bias=bt3[:, 0:1])
                outv = out_d[grp * 96 : (grp + 1) * 96, :].rearrange(
                    "(b q) p -> q b p", b=4)
                nc.sync.dma_start(
                    out=outv, in_=s3[:].rearrange("q (b p) -> q b p", b=4))

    nc.compile()
    consts = dict(W1bd=W1bd, W2bd=W2bd, W3bd=W3bd, b1t=b1t, b2t=b2t, b3t=b3t)
    return nc, consts


def kernel(x, edge_index, edge_attr, u, batch, W1, b1, W2, b2, W3, b3):
    import sys, types

    try:
        import antenv.axon_hooks  # noqa: F401
    except ImportError:
        try:
            import antenv
            mod = types.ModuleType("antenv.axon_hooks")
            _H = [None]
            mod.set_axon_ntff_profile_hook = lambda h: _H.__setitem__(0, h)
            mod.get_axon_ntff_profile_hook = lambda: _H[0]
            sys.modules["antenv.axon_hooks"] = mod
            antenv.axon_hooks = mod
            from trn_agent_boot.trn_boot import _ntff_profile_via_ctypes
            hook = _ntff_profile_via_ctypes("/opt/axon/libaxon_pjrt.so")
            if hook is not None:
                mod.set_axon_ntff_profile_hook(hook)
        except Exception:
            pass

    from concourse.bass_utils import run_bass_kernel_spmd

    x = np.asarray(x, np.float32)
    row = np.asarray(edge_index[0]).astype(np.int64)
    col = np.asarray(edge_index[1]).astype(np.int64)
    W1 = np.asarray(W1, np.float32); b1 = np.asarray(b1, np.float32)
    W2 = np.asarray(W2, np.float32); b2 = np.asarray(b2, np.float32)
    W3 = np.asarray(W3, np.float32); b3 = np.asarray(b3, np.float32)

    prep = _host_prep(x, row, col)
    nc, consts = _build_program(prep, W1, b1, W2, b2, W3, b3)

    J = prep["J"]
    in_maps = []
    for k in range(N_CORES):
        in_maps.append({
            "x_dev": prep["x_dev"],
            "offs": prep["streams"][k],
            "h0": prep["x_shard6"][k].reshape(P, J * 6),
            "W1bd": consts["W1bd"], "W2bd": consts["W2bd"], "W3bd": consts["W3bd"],
            "b1t": consts["b1t"], "b2t": consts["b2t"], "b3t": consts["b3t"],
        })
    import os
    _tr = os.environ.get("KERNEL_TRACE") == "1"
    res = run_bass_kernel_spmd(nc, in_maps, list(range(N_CORES)), trace=_tr)
    if _tr:
        kernel._last_exec_ns = res.exec_time_ns
    outs = np.stack([res.results[k]["out"].reshape(J, 3, P) for k in range(N_CORES)])
    full = outs[prep["node_core"], prep["node_j"], :, prep["node_p"]]
    kernel._last_nc = nc
    kernel._last_in_maps = in_maps
    return full.astype(np.float32)
